# revision 12
# baseline (speedup 1.0000x reference)
"""Trainium2 Bass kernel for CharOffsetAttention (RoPE attention block).

Sharding (8 cores): data-parallel over batch (B=4 -> pairs of cores) x
tensor-parallel over heads (16 heads -> 8 per core).  Each core computes
qkv projections for its 8 heads, rope, causal attention, and a partial
output projection; the host sums the two head-half partials per batch.

v3 schedule: single software-pipelined stream.  Attention runs h-major
(q 512-blocks outer, head-pairs inner); the q/k/v projection matmuls are
split into 512-column chunks and fed into the attention stream as PE
"filler" between score/PV groups so the tensor engine never idles long
enough to trip the HAM clock gate (idle >3.4us -> PE drops to 1.2GHz).
Scores for both heads of a pair go to one [128,1024] PSUM tile so ONE
ScalarE exp serves both heads (halves the per-instruction overhead).
Softmax denominators (ones-column of v) are packed into 32-aligned
partition slots and hit with one reciprocal per pair; the broadcast
back across head-dims runs on gpsimd (partition_broadcast, attn
library), keeping the whole epilogue off the PE queue.
"""
import sys

if '/opt/trn_rl_repo' not in sys.path:
    sys.path.insert(0, '/opt/trn_rl_repo')

import numpy as np
import ml_dtypes

import concourse.bass as bass
import concourse.bacc as bacc
import concourse.tile as tile
import concourse.mybir as mybir
from concourse import library_config
from concourse.bass_utils import run_bass_kernel_spmd

F32 = mybir.dt.float32
BF16 = mybir.dt.bfloat16
NPBF16 = ml_dtypes.bfloat16

# full-problem constants
B, T, D_MODEL, N_HEADS, HEAD_DIM = 4, 2048, 1024, 16, 64
N_CORES = 8


class FillerQueue:
    """Dependency-free PE work units drained between attention groups.

    Units are (due_slot, fn) kept in FIFO order (callers add in due
    order); `step(n)` emits up to n units, `drain(slot)` force-emits
    everything due at or before `slot`.
    """

    def __init__(self):
        self.units = []
        self.head = 0

    def add(self, due, fn):
        self.units.append((due, fn))

    def step(self, n):
        for _ in range(n):
            if self.head >= len(self.units):
                return
            self.units[self.head][1]()
            self.head += 1

    def drain(self, slot):
        while self.head < len(self.units) and \
                self.units[self.head][0] <= slot:
            self.units[self.head][1]()
            self.head += 1

    def drain_all(self):
        self.drain(1 << 30)


def build_program(t=T, din=D_MODEL, nhc=N_HEADS // 2, debug=False):
    """Build the per-core SPMD program."""
    hd = HEAD_DIM
    dh = nhc * hd                    # head dims handled by this core
    nkc = t // 128                   # k-token chunks
    nh = t // 512                    # q 512-blocks (attention rounds)
    npair = nhc // 2
    ndin = din // 128
    scale = 1.0 / np.sqrt(hd)

    nc = bacc.Bacc("TRN2", target_bir_lowering=False, debug=debug,
                   num_devices=N_CORES)

    xT_d = nc.dram_tensor("xT", [din, t], BF16, kind="ExternalInput")
    wqT_d = nc.dram_tensor("wqT", [din, dh], BF16, kind="ExternalInput")
    wkT_d = nc.dram_tensor("wkT", [din, dh], BF16, kind="ExternalInput")
    wvT_d = nc.dram_tensor("wvT", [din, dh], BF16, kind="ExternalInput")
    woT_d = nc.dram_tensor("woT", [dh, din], BF16, kind="ExternalInput")
    cos_d = nc.dram_tensor("cosr", [128, t], BF16, kind="ExternalInput")
    sinp_d = nc.dram_tensor("sinp", [128, t], BF16, kind="ExternalInput")
    sinn_d = nc.dram_tensor("sinn", [128, t], BF16, kind="ExternalInput")
    out_d = nc.dram_tensor("out", [t, din], F32, kind="ExternalOutput")

    with tile.TileContext(nc) as tc:
        with (
            tc.tile_pool(name="persist", bufs=1) as pp,
            tc.tile_pool(name="pin", bufs=1) as pin,
        ):
            # ones rows for the denominator broadcast matmuls (full
            # 128 partitions so any 32-aligned slice matches the rec
            # row's base partition - matmul requires equal bases)
            ones1 = pp.tile([128, 64], BF16, tag="ones1")
            nc.gpsimd.memset(ones1[:], 1.0)

            # ---- persistent SBUF tensors ----
            cos_sb = pp.tile([128, t], BF16, tag="cos")
            sinp_sb = pp.tile([128, t], BF16, tag="sinp")
            sinn_sb = pp.tile([128, t], BF16, tag="sinn")
            woT_sb = [pp.tile([128, din], BF16, tag=f"woT{i}",
                              name=f"woT{i}") for i in range(dh // 128)]
            qT = [pp.tile([128, t], BF16, tag=f"qT{c}", name=f"qT{c}")
                  for c in range(npair)]
            kT = [pp.tile([128, t], BF16, tag=f"kT{c}", name=f"kT{c}")
                  for c in range(npair)]
            v_sb = [pp.tile([128, nhc * (hd + 1)], BF16, tag=f"v{i}",
                            name=f"v{i}") for i in range(nkc)]
            attT = [pp.tile([128, t], BF16, tag=f"attT{c}",
                            name=f"attT{c}") for c in range(npair)]
            # softmax denominators, 8 slots per pair: slot r=(2*h+ab)
            # at partition 32*(r%3), cols [512*(r//3) : +512) -- matmul
            # operand base partitions must be in {0, 32, 64}
            den_sb = [pp.tile([128, 1536], F32, tag=f"den{c}",
                              name=f"den{c}") for c in range(npair)]
            for c in range(npair):
                nc.gpsimd.memset(den_sb[c][:], 1.0)

            # ---- staged inputs ----
            xT_sb = [pin.tile([128, t], BF16, tag=f"xT{k}",
                              name=f"xTs{k}") for k in range(ndin)]
            wqT_sb = [pin.tile([128, dh], BF16, tag=f"wqT{k}",
                               name=f"wqTs{k}") for k in range(ndin)]
            wkT_sb = [pin.tile([128, dh], BF16, tag=f"wkT{k}",
                               name=f"wkTs{k}") for k in range(ndin)]
            wvT_sb = [pin.tile([128, dh], BF16, tag=f"wvT{k}",
                               name=f"wvTs{k}") for k in range(ndin)]
            # DMA order: small rope tables first, then x/w interleaved
            # per din-chunk so the kk-accumulating projection matmuls
            # start ~2us in and pipeline behind the DMA.
            nc.sync.dma_start(cos_sb[:], cos_d[:])
            nc.sync.dma_start(sinp_sb[:], sinp_d[:])
            nc.sync.dma_start(sinn_sb[:], sinn_d[:])
            for k in range(ndin):
                sl = slice(k * 128, (k + 1) * 128)
                nc.sync.dma_start(xT_sb[k][:], xT_d[sl, :])
                nc.sync.dma_start(wqT_sb[k][:], wqT_d[sl, :])
                nc.sync.dma_start(wkT_sb[k][:], wkT_d[sl, :])
                nc.sync.dma_start(wvT_sb[k][:], wvT_d[sl, :])
            for i in range(dh // 128):
                nc.sync.dma_start(woT_sb[i][:],
                                  woT_d[i * 128:(i + 1) * 128, :])

            with (
                tc.tile_pool(name="projps", bufs=2, space="PSUM") as pjp,
                tc.tile_pool(name="stps", bufs=2, space="PSUM") as stp,
                tc.tile_pool(name="attps", bufs=2, space="PSUM") as atp,
                tc.tile_pool(name="praw", bufs=2) as praw,
                tc.tile_pool(name="rtmp", bufs=2) as rtp,
                tc.tile_pool(name="exps", bufs=4) as exps,
                tc.tile_pool(name="recs", bufs=2) as rcp,
            ):
                # ---------- filler thunks: projections in 512 chunks ----
                filler = FillerQueue()

                def rope(raw, dst, n0):
                    """rope on a 512-col chunk; column-local."""
                    cs = slice(n0, n0 + 512)
                    tmp = rtp.tile([128, 512], BF16, tag="rtmp")
                    nc.vector.tensor_mul(tmp[0:32], raw[32:64],
                                         sinn_sb[32:64, cs])
                    nc.vector.tensor_mul(tmp[32:64], raw[0:32],
                                         sinp_sb[0:32, cs])
                    nc.vector.tensor_mul(tmp[64:96], raw[96:128],
                                         sinn_sb[96:128, cs])
                    nc.vector.tensor_mul(tmp[96:128], raw[64:96],
                                         sinp_sb[64:96, cs])
                    nc.vector.tensor_mul(dst[:], raw[:], cos_sb[:, cs])
                    nc.vector.tensor_add(dst[:], dst[:], tmp[:])

                def add_qk_chunk(w_sb, dst, c, n):
                    due = n * npair + c
                    state = {}

                    def mm(kk):
                        def fn():
                            if kk == 0:
                                state['ps'] = pjp.tile(
                                    [128, 512], F32, tag="pjp",
                                    name=f"pjq{c}_{n}")
                            nc.tensor.matmul(
                                state['ps'][:],
                                w_sb[kk][:, c * 128:(c + 1) * 128],
                                xT_sb[kk][:, n * 512:(n + 1) * 512],
                                start=(kk == 0), stop=(kk == ndin - 1))
                        return fn

                    def fin():
                        raw = praw.tile([128, 512], BF16, tag="praw")
                        nc.vector.tensor_copy(raw[:], state['ps'][:])
                        rope(raw, dst[:, n * 512:(n + 1) * 512], n * 512)
                    for kk in range(ndin):
                        filler.add(due, mm(kk))
                    filler.add(due, fin)

                def add_v_chunk(tt):
                    due = (tt // 4) * npair
                    state = {}

                    def mm(kk):
                        def fn():
                            if kk == 0:
                                state['ps'] = pjp.tile(
                                    [128, 512], F32, tag="pjp",
                                    name=f"pjv{tt}")
                            nc.tensor.matmul(
                                state['ps'][:],
                                xT_sb[kk][:, tt * 128:(tt + 1) * 128],
                                wvT_sb[kk][:],
                                start=(kk == 0), stop=(kk == ndin - 1))
                        return fn

                    def fin():
                        nc.gpsimd.memset(v_sb[tt][:], 1.0)
                        vdst = v_sb[tt].rearrange("p (h e) -> p h e",
                                                  h=nhc)
                        vsrc = state['ps'].rearrange("p (h e) -> p h e",
                                                     h=nhc)
                        nc.vector.tensor_copy(vdst[:, :, 0:hd], vsrc[:])
                    for kk in range(ndin):
                        filler.add(due, mm(kk))
                    filler.add(due, fin)

                # enqueue strictly in due order (drain() walks a FIFO
                # prefix): pair-0 q/k of round n, then round-n v chunks
                # (all due at slot n*npair), then the later pairs
                for n in range(nh):
                    add_qk_chunk(wqT_sb, qT[0], 0, n)
                    add_qk_chunk(wkT_sb, kT[0], 0, n)
                    for tt in range(4 * n, 4 * n + 4):
                        add_v_chunk(tt)
                    for c in range(1, npair):
                        add_qk_chunk(wqT_sb, qT[c], c, n)
                        add_qk_chunk(wkT_sb, kT[c], c, n)

                # ---- deferred epilogue part 2: one reciprocal, PE
                # ones-matmul broadcasts (bc tiles reuse the projection
                # PSUM pool), in-place DVE normalize muls.  Emitted one
                # slot late so the PE queue never waits on the recip.
                def part2(pr):
                    rec = rcp.tile([128, 1536], F32, tag="rec")
                    nc.vector.reciprocal_approx_fast(rec[:],
                                                     den_sb[pr][:])
                    recb = rcp.tile([128, 1536], BF16, tag="recb")
                    nc.vector.tensor_copy(recb[:], rec[:])
                    for nq in range(nh):
                        bc = pjp.tile([128, 512], F32, tag="pjp",
                                      name=f"bc{pr}_{nq}")
                        for ab in range(2):
                            r = 2 * nq + ab
                            rp, rc0 = 32 * (r % 3), 512 * (r // 3)
                            nc.tensor.matmul(
                                bc[ab * 64:(ab + 1) * 64, :],
                                ones1[rp:rp + 1, :],
                                recb[rp:rp + 1, rc0:rc0 + 512],
                                start=True, stop=True)
                        sl = slice(nq * 512, (nq + 1) * 512)
                        nc.vector.tensor_mul(
                            attT[pr][:, sl], attT[pr][:, sl], bc[:])

                # ---------- attention: h-major, pairs inner ----------
                for h in range(nh):
                    h0 = h * 512
                    # last round runs pair 3 first so its normalization
                    # (part2) is done before phase 3 needs attT[3]
                    pr_seq = list(range(npair)) if h < nh - 1 else \
                        [npair - 1] + list(range(npair - 1))
                    for pri, pr in enumerate(pr_seq):
                        slot = h * npair + pr
                        filler.drain(slot)
                        att = [atp.tile([hd + 1, 512], F32, tag="attps",
                                        name=f"att{slot}_{ab}")
                               for ab in range(2)]
                        last_kc = 4 * h + 3

                        def emit_pv(kc, ex):
                            # columns left of the causal diagonal are
                            # all-zero in ex; skip them
                            off = 128 * max(0, kc - 4 * h)
                            for ab in range(2):
                                hh = 2 * pr + ab
                                nc.tensor.matmul(
                                    att[ab][:, off:512],
                                    v_sb[kc][:, hh * (hd + 1):
                                             (hh + 1) * (hd + 1)],
                                    ex[:, ab * 512 + off:(ab + 1) * 512],
                                    start=(kc == 0),
                                    stop=(kc == last_kc))

                        pending = None
                        for kc in range(last_kc + 1):
                            st = stp.tile([128, 1024], F32, tag="st")
                            for ab in range(2):
                                po = ab * 64
                                nc.tensor.matmul(
                                    st[:, ab * 512:(ab + 1) * 512],
                                    kT[pr][po:po + 64,
                                           kc * 128:(kc + 1) * 128],
                                    qT[pr][po:po + 64, h0:h0 + 512],
                                    start=True, stop=True)
                            ex = exps.tile([128, 1024], BF16, tag="ex")
                            nc.scalar.activation(
                                ex[:], st[:],
                                mybir.ActivationFunctionType.Exp,
                                scale=float(scale))
                            if kc // 4 == h:
                                for ab in range(2):
                                    sl = slice(ab * 512, ab * 512 + 512)
                                    nc.gpsimd.affine_select(
                                        out=ex[:, sl], in_=ex[:, sl],
                                        compare_op=mybir.AluOpType.is_ge,
                                        fill=0.0, base=h0 - 128 * kc,
                                        pattern=[[1, 512]],
                                        channel_multiplier=-1)
                            filler.step(3)
                            if pending is not None:
                                emit_pv(*pending)
                            pending = (kc, ex)
                        emit_pv(*pending)

                        # part 2 of the PREVIOUS pair, now that a full
                        # slot of PE work is queued ahead of it
                        if h == nh - 1 and pri > 0:
                            part2(pr_seq[pri - 1])

                        # epilogue part 1: evict PSUM (unnormalized) and
                        # pack the denominator row into its 32-aligned slot
                        for ab in range(2):
                            r = 2 * h + ab
                            rp, rc0 = 32 * (r % 3), 512 * (r // 3)
                            nc.vector.tensor_copy(
                                attT[pr][ab * 64:(ab + 1) * 64,
                                         h0:h0 + 512],
                                att[ab][0:hd, :])
                            nc.vector.tensor_copy(
                                den_sb[pr][rp:rp + 1, rc0:rc0 + 512],
                                att[ab][hd:hd + 1, :])
                filler.drain_all()
                part2(npair - 2)

            # =========== phase 3: output projection (partial) ===========
            with (
                tc.tile_pool(name="wops", bufs=2, space="PSUM") as wop,
                tc.tile_pool(name="outsb", bufs=3) as osb,
            ):
                kks = [dh // 128 - 1] + list(range(dh // 128 - 1))
                for tt in range(nkc):
                    ps = wop.tile([128, min(din, 1024)], F32, tag="wops")
                    for i, kk in enumerate(kks):
                        lhsT = attT[kk][:, tt * 128:(tt + 1) * 128]
                        for n2 in range(0, din, 512):
                            nc.tensor.matmul(
                                ps[:, n2:n2 + 512], lhsT,
                                woT_sb[kk][:, n2:n2 + 512],
                                start=(i == 0), stop=(i == len(kks) - 1))
                    o = osb.tile([128, min(din, 1024)], F32, tag="outsb")
                    nc.scalar.copy(o[:], ps[:])
                    nc.sync.dma_start(out_d[tt * 128:(tt + 1) * 128, :], o[:])

    nc.compile()
    return nc


_PROG = None


def _get_program():
    global _PROG
    if _PROG is None:
        _PROG = build_program()
    return _PROG


def _rope_perm(nhc):
    """Per-head row permutation deinterleaving (re, im) pairs."""
    p = []
    for h in range(nhc):
        base = h * HEAD_DIM
        p.extend(base + 2 * i for i in range(HEAD_DIM // 2))
        p.extend(base + 2 * i + 1 for i in range(HEAD_DIM // 2))
    return np.array(p)


def make_core_inputs(x, position_ids, wq, wk, wv, wo, freqs_cos, freqs_sin):
    """Shard + pre-layout the full inputs for the 8 cores."""
    fc = np.asarray(freqs_cos, np.float32)
    fs = np.asarray(freqs_sin, np.float32)
    pos = np.asarray(position_ids)
    perm = _rope_perm(N_HEADS)
    wq_p = np.asarray(wq, np.float32)[perm]
    wk_p = np.asarray(wk, np.float32)[perm]
    wv_ = np.asarray(wv, np.float32)
    wo_ = np.asarray(wo, np.float32)

    in_maps = []
    for c in range(N_CORES):
        b, hh = c // 2, c % 2
        hs = slice(hh * 8 * HEAD_DIM, (hh + 1) * 8 * HEAD_DIM)
        cos_b = fc[pos[b]]                    # [T, 32]
        sin_b = fs[pos[b]]
        cosr = np.tile(cos_b.T, (4, 1))       # [128, T]
        sinr = np.tile(sin_b.T, (4, 1))
        in_maps.append({
            "xT": np.ascontiguousarray(
                np.asarray(x[b], np.float32).T).astype(NPBF16),
            "wqT": np.ascontiguousarray(wq_p[hs].T).astype(NPBF16),
            "wkT": np.ascontiguousarray(wk_p[hs].T).astype(NPBF16),
            "wvT": np.ascontiguousarray(wv_[hs].T).astype(NPBF16),
            "woT": np.ascontiguousarray(wo_[:, hs].T).astype(NPBF16),
            "cosr": cosr.astype(NPBF16),
            "sinp": sinr.astype(NPBF16),
            "sinn": (-sinr).astype(NPBF16),
        })
    return in_maps


def kernel(x, position_ids, mask, wq, wk, wv, wo, freqs_cos, freqs_sin,
           trace=False):
    nc = _get_program()
    in_maps = make_core_inputs(x, position_ids, wq, wk, wv, wo,
                               freqs_cos, freqs_sin)
    res = run_bass_kernel_spmd(nc, in_maps, list(range(N_CORES)),
                               trace=trace, trace_cores=[0] if trace else None)
    outs = [res.results[c]["out"] for c in range(N_CORES)]
    full = np.stack([outs[2 * b] + outs[2 * b + 1] for b in range(B)])
    kernel.last_results = res
    return full.astype(np.float32)


# revision 13
# speedup vs baseline: 1.0142x; 1.0142x over previous
"""Trainium2 Bass kernel for CharOffsetAttention (RoPE attention block).

Sharding (8 cores): data-parallel over batch (B=4 -> pairs of cores) x
tensor-parallel over heads (16 heads -> 8 per core).  Each core computes
qkv projections for its 8 heads, rope, causal attention, and a partial
output projection; the host sums the two head-half partials per batch.

v3 schedule: single software-pipelined stream.  Attention runs h-major
(q 512-blocks outer, head-pairs inner); the q/k/v projection matmuls are
split into 512-column chunks and fed into the attention stream as PE
"filler" between score/PV groups so the tensor engine never idles long
enough to trip the HAM clock gate (idle >3.4us -> PE drops to 1.2GHz).
Scores for both heads of a pair go to one [128,1024] PSUM tile so ONE
ScalarE exp serves both heads (halves the per-instruction overhead).
Softmax denominators (ones-column of v) are packed into 32-aligned
partition slots and hit with one reciprocal per pair; the broadcast
back across head-dims runs on gpsimd (partition_broadcast, attn
library), keeping the whole epilogue off the PE queue.
"""
import sys

if '/opt/trn_rl_repo' not in sys.path:
    sys.path.insert(0, '/opt/trn_rl_repo')

import numpy as np
import ml_dtypes

import concourse.bass as bass
import concourse.bacc as bacc
import concourse.tile as tile
import concourse.mybir as mybir
from concourse import library_config
from concourse.bass_utils import run_bass_kernel_spmd

F32 = mybir.dt.float32
BF16 = mybir.dt.bfloat16
NPBF16 = ml_dtypes.bfloat16

# full-problem constants
B, T, D_MODEL, N_HEADS, HEAD_DIM = 4, 2048, 1024, 16, 64
N_CORES = 8


class FillerQueue:
    """Dependency-free PE work units drained between attention groups.

    Units are (due_slot, fn) kept in FIFO order (callers add in due
    order); `step(n)` emits up to n units, `drain(slot)` force-emits
    everything due at or before `slot`.
    """

    def __init__(self):
        self.units = []
        self.head = 0

    def add(self, due, fn):
        self.units.append((due, fn))

    def step(self, n):
        for _ in range(n):
            if self.head >= len(self.units):
                return
            self.units[self.head][1]()
            self.head += 1

    def drain(self, slot):
        while self.head < len(self.units) and \
                self.units[self.head][0] <= slot:
            self.units[self.head][1]()
            self.head += 1

    def drain_all(self):
        self.drain(1 << 30)


def build_program(t=T, din=D_MODEL, nhc=N_HEADS // 2, debug=False):
    """Build the per-core SPMD program."""
    hd = HEAD_DIM
    dh = nhc * hd                    # head dims handled by this core
    nkc = t // 128                   # k-token chunks
    nh = t // 512                    # q 512-blocks (attention rounds)
    npair = nhc // 2
    ndin = din // 128
    scale = 1.0 / np.sqrt(hd)

    nc = bacc.Bacc("TRN2", target_bir_lowering=False, debug=debug,
                   num_devices=N_CORES)

    xT_d = nc.dram_tensor("xT", [din, t], BF16, kind="ExternalInput")
    wqT_d = nc.dram_tensor("wqT", [din, dh], BF16, kind="ExternalInput")
    wkT_d = nc.dram_tensor("wkT", [din, dh], BF16, kind="ExternalInput")
    wvT_d = nc.dram_tensor("wvT", [din, dh], BF16, kind="ExternalInput")
    woT_d = nc.dram_tensor("woT", [dh, din], BF16, kind="ExternalInput")
    cos_d = nc.dram_tensor("cosr", [128, t], BF16, kind="ExternalInput")
    sinp_d = nc.dram_tensor("sinp", [128, t], BF16, kind="ExternalInput")
    sinn_d = nc.dram_tensor("sinn", [128, t], BF16, kind="ExternalInput")
    out_d = nc.dram_tensor("out", [t, din], F32, kind="ExternalOutput")

    with tile.TileContext(nc) as tc:
        with (
            tc.tile_pool(name="persist", bufs=1) as pp,
            tc.tile_pool(name="pin", bufs=1) as pin,
        ):
            # ones rows for the denominator broadcast matmuls (full
            # 128 partitions so any 32-aligned slice matches the rec
            # row's base partition - matmul requires equal bases)
            ones1 = pp.tile([128, 64], BF16, tag="ones1")
            nc.gpsimd.memset(ones1[:], 1.0)

            # ---- persistent SBUF tensors ----
            cos_sb = pp.tile([128, t], BF16, tag="cos")
            sinp_sb = pp.tile([128, t], BF16, tag="sinp")
            sinn_sb = pp.tile([128, t], BF16, tag="sinn")
            woT_sb = [pp.tile([128, din], BF16, tag=f"woT{i}",
                              name=f"woT{i}") for i in range(dh // 128)]
            qT = [pp.tile([128, t], BF16, tag=f"qT{c}", name=f"qT{c}")
                  for c in range(npair)]
            kT = [pp.tile([128, t], BF16, tag=f"kT{c}", name=f"kT{c}")
                  for c in range(npair)]
            v_sb = [pp.tile([128, nhc * (hd + 1)], BF16, tag=f"v{i}",
                            name=f"v{i}") for i in range(nkc)]
            attT = [pp.tile([128, t], BF16, tag=f"attT{c}",
                            name=f"attT{c}") for c in range(npair)]
            # softmax denominators, 8 slots per pair: slot r=(2*h+ab)
            # at partition 32*(r%3), cols [512*(r//3) : +512) -- matmul
            # operand base partitions must be in {0, 32, 64}
            den_sb = [pp.tile([128, 1536], F32, tag=f"den{c}",
                              name=f"den{c}") for c in range(npair)]
            for c in range(npair):
                nc.gpsimd.memset(den_sb[c][:], 1.0)

            # ---- staged inputs ----
            xT_sb = [pin.tile([128, t], BF16, tag=f"xT{k}",
                              name=f"xTs{k}") for k in range(ndin)]
            wqT_sb = [pin.tile([128, dh], BF16, tag=f"wqT{k}",
                               name=f"wqTs{k}") for k in range(ndin)]
            wkT_sb = [pin.tile([128, dh], BF16, tag=f"wkT{k}",
                               name=f"wkTs{k}") for k in range(ndin)]
            wvT_sb = [pin.tile([128, dh], BF16, tag=f"wvT{k}",
                               name=f"wvTs{k}") for k in range(ndin)]
            # DMA: rope tables + wo ride the Activation hwdge queue in
            # parallel with the main x/w stream on the sync queue; the
            # main stream is grouped (x, wq, wk, wv) so the pair-0 q/k
            # projections complete as early as possible.
            nc.scalar.dma_start(cos_sb[:], cos_d[:])
            nc.scalar.dma_start(sinp_sb[:], sinp_d[:])
            nc.scalar.dma_start(sinn_sb[:], sinn_d[:])
            for i in range(dh // 128):
                nc.scalar.dma_start(woT_sb[i][:],
                                    woT_d[i * 128:(i + 1) * 128, :])
            for k in range(ndin):
                nc.sync.dma_start(xT_sb[k][:],
                                  xT_d[k * 128:(k + 1) * 128, :])
            for k in range(ndin):
                nc.sync.dma_start(wqT_sb[k][:],
                                  wqT_d[k * 128:(k + 1) * 128, :])
            for k in range(ndin):
                nc.sync.dma_start(wkT_sb[k][:],
                                  wkT_d[k * 128:(k + 1) * 128, :])
            for k in range(ndin):
                nc.sync.dma_start(wvT_sb[k][:],
                                  wvT_d[k * 128:(k + 1) * 128, :])

            with (
                tc.tile_pool(name="projps", bufs=2, space="PSUM") as pjp,
                tc.tile_pool(name="stps", bufs=2, space="PSUM") as stp,
                tc.tile_pool(name="attps", bufs=2, space="PSUM") as atp,
                tc.tile_pool(name="praw", bufs=2) as praw,
                tc.tile_pool(name="rtmp", bufs=2) as rtp,
                tc.tile_pool(name="exps", bufs=4) as exps,
                tc.tile_pool(name="recs", bufs=2) as rcp,
            ):
                # ---------- filler thunks: projections in 512 chunks ----
                filler = FillerQueue()

                def rope(raw, dst, n0):
                    """rope on a 512-col chunk; column-local."""
                    cs = slice(n0, n0 + 512)
                    tmp = rtp.tile([128, 512], BF16, tag="rtmp")
                    nc.vector.tensor_mul(tmp[0:32], raw[32:64],
                                         sinn_sb[32:64, cs])
                    nc.vector.tensor_mul(tmp[32:64], raw[0:32],
                                         sinp_sb[0:32, cs])
                    nc.vector.tensor_mul(tmp[64:96], raw[96:128],
                                         sinn_sb[96:128, cs])
                    nc.vector.tensor_mul(tmp[96:128], raw[64:96],
                                         sinp_sb[64:96, cs])
                    nc.vector.tensor_mul(dst[:], raw[:], cos_sb[:, cs])
                    nc.vector.tensor_add(dst[:], dst[:], tmp[:])

                def seq_pos(n, c):
                    # emission sequence position of slot (h=n, pr=c):
                    # the last round runs pair npair-1 first
                    if n < nh - 1:
                        return n * npair + c
                    order = [npair - 1] + list(range(npair - 1))
                    return n * npair + order.index(c)

                def add_qk_chunk(w_sb, dst, c, n):
                    due = seq_pos(n, c)
                    state = {}

                    def mm(kk):
                        def fn():
                            if kk == 0:
                                state['ps'] = pjp.tile(
                                    [128, 512], F32, tag="pjp",
                                    name=f"pjq{c}_{n}")
                            nc.tensor.matmul(
                                state['ps'][:],
                                w_sb[kk][:, c * 128:(c + 1) * 128],
                                xT_sb[kk][:, n * 512:(n + 1) * 512],
                                start=(kk == 0), stop=(kk == ndin - 1))
                        return fn

                    def fin():
                        raw = praw.tile([128, 512], BF16, tag="praw")
                        nc.vector.tensor_copy(raw[:], state['ps'][:])
                        rope(raw, dst[:, n * 512:(n + 1) * 512], n * 512)
                    for kk in range(ndin):
                        filler.add(due, mm(kk))
                    filler.add(due, fin)

                def add_v_chunk(tt):
                    due = (tt // 4) * npair
                    state = {}

                    def mm(kk):
                        def fn():
                            if kk == 0:
                                state['ps'] = pjp.tile(
                                    [128, 512], F32, tag="pjp",
                                    name=f"pjv{tt}")
                            nc.tensor.matmul(
                                state['ps'][:],
                                xT_sb[kk][:, tt * 128:(tt + 1) * 128],
                                wvT_sb[kk][:],
                                start=(kk == 0), stop=(kk == ndin - 1))
                        return fn

                    def fin():
                        nc.gpsimd.memset(v_sb[tt][:], 1.0)
                        vdst = v_sb[tt].rearrange("p (h e) -> p h e",
                                                  h=nhc)
                        vsrc = state['ps'].rearrange("p (h e) -> p h e",
                                                     h=nhc)
                        nc.vector.tensor_copy(vdst[:, :, 0:hd], vsrc[:])
                    for kk in range(ndin):
                        filler.add(due, mm(kk))
                    filler.add(due, fin)

                # enqueue strictly in due order (drain() walks a FIFO
                # prefix): pair-0 q/k of round n, then round-n v chunks
                # (all due at slot n*npair), then the later pairs
                for n in range(nh):
                    add_qk_chunk(wqT_sb, qT[0], 0, n)
                    add_qk_chunk(wkT_sb, kT[0], 0, n)
                    for tt in range(4 * n, 4 * n + 4):
                        add_v_chunk(tt)
                    for c in range(1, npair):
                        add_qk_chunk(wqT_sb, qT[c], c, n)
                        add_qk_chunk(wkT_sb, kT[c], c, n)

                # ---- deferred epilogue part 2: one reciprocal, PE
                # ones-matmul broadcasts (bc tiles reuse the projection
                # PSUM pool), in-place DVE normalize muls.  Emitted one
                # slot late so the PE queue never waits on the recip.
                def part2(pr):
                    rec = rcp.tile([128, 1536], F32, tag="rec")
                    nc.vector.reciprocal_approx_fast(rec[:],
                                                     den_sb[pr][:])
                    recb = rcp.tile([128, 1536], BF16, tag="recb")
                    nc.vector.tensor_copy(recb[:], rec[:])
                    for nq in range(nh):
                        bc = pjp.tile([128, 512], F32, tag="pjp",
                                      name=f"bc{pr}_{nq}")
                        for ab in range(2):
                            r = 2 * nq + ab
                            rp, rc0 = 32 * (r % 3), 512 * (r // 3)
                            nc.tensor.matmul(
                                bc[ab * 64:(ab + 1) * 64, :],
                                ones1[rp:rp + 1, :],
                                recb[rp:rp + 1, rc0:rc0 + 512],
                                start=True, stop=True)
                        sl = slice(nq * 512, (nq + 1) * 512)
                        nc.vector.tensor_mul(
                            attT[pr][:, sl], attT[pr][:, sl], bc[:])

                # ---------- attention: h-major, pairs inner ----------
                for h in range(nh):
                    h0 = h * 512
                    # last round runs pair 3 first so its normalization
                    # (part2) is done before phase 3 needs attT[3]
                    pr_seq = list(range(npair)) if h < nh - 1 else \
                        [npair - 1] + list(range(npair - 1))
                    for pri, pr in enumerate(pr_seq):
                        slot = h * npair + pri
                        filler.drain(slot)
                        att = [atp.tile([hd + 1, 512], F32, tag="attps",
                                        name=f"att{slot}_{ab}")
                               for ab in range(2)]
                        last_kc = 4 * h + 3

                        def emit_pv(kc, ex):
                            # columns left of the causal diagonal are
                            # all-zero in ex; skip them
                            off = 128 * max(0, kc - 4 * h)
                            for ab in range(2):
                                hh = 2 * pr + ab
                                nc.tensor.matmul(
                                    att[ab][:, off:512],
                                    v_sb[kc][:, hh * (hd + 1):
                                             (hh + 1) * (hd + 1)],
                                    ex[:, ab * 512 + off:(ab + 1) * 512],
                                    start=(kc == 0),
                                    stop=(kc == last_kc))

                        pending = None
                        for kc in range(last_kc + 1):
                            # columns left of the causal diagonal are
                            # never consumed: trim scores + exp to the
                            # valid region [off, 512) of each head
                            off = 128 * max(0, kc - 4 * h)
                            st = stp.tile([128, 1024], F32, tag="st")
                            for ab in range(2):
                                po = ab * 64
                                nc.tensor.matmul(
                                    st[:, ab * 512 + off:
                                       (ab + 1) * 512],
                                    kT[pr][po:po + 64,
                                           kc * 128:(kc + 1) * 128],
                                    qT[pr][po:po + 64,
                                           h0 + off:h0 + 512],
                                    start=True, stop=True)
                            ex = exps.tile([128, 1024], BF16, tag="ex")
                            if off == 0:
                                nc.scalar.activation(
                                    ex[:], st[:],
                                    mybir.ActivationFunctionType.Exp,
                                    scale=float(scale))
                            else:
                                for ab in range(2):
                                    sl = slice(ab * 512 + off,
                                               (ab + 1) * 512)
                                    nc.scalar.activation(
                                        ex[:, sl], st[:, sl],
                                        mybir.ActivationFunctionType.Exp,
                                        scale=float(scale))
                            if kc // 4 == h:
                                # triangle mask on the 128-col diagonal
                                # block only
                                for ab in range(2):
                                    sl = slice(ab * 512 + off,
                                               ab * 512 + off + 128)
                                    nc.gpsimd.affine_select(
                                        out=ex[:, sl], in_=ex[:, sl],
                                        compare_op=mybir.AluOpType.is_ge,
                                        fill=0.0, base=0,
                                        pattern=[[1, 128]],
                                        channel_multiplier=-1)
                            filler.step(3)
                            if pending is not None:
                                emit_pv(*pending)
                            pending = (kc, ex)
                        emit_pv(*pending)

                        # part 2 of the PREVIOUS pair, now that a full
                        # slot of PE work is queued ahead of it
                        if h == nh - 1 and pri > 0:
                            part2(pr_seq[pri - 1])

                        # epilogue part 1: evict PSUM (unnormalized) and
                        # pack the denominator row into its 32-aligned slot
                        for ab in range(2):
                            r = 2 * h + ab
                            rp, rc0 = 32 * (r % 3), 512 * (r // 3)
                            nc.vector.tensor_copy(
                                attT[pr][ab * 64:(ab + 1) * 64,
                                         h0:h0 + 512],
                                att[ab][0:hd, :])
                            nc.vector.tensor_copy(
                                den_sb[pr][rp:rp + 1, rc0:rc0 + 512],
                                att[ab][hd:hd + 1, :])
                filler.drain_all()
                part2(npair - 2)

            # =========== phase 3: output projection (partial) ===========
            with (
                tc.tile_pool(name="wops", bufs=2, space="PSUM") as wop,
                tc.tile_pool(name="outsb", bufs=3) as osb,
            ):
                kks = [dh // 128 - 1] + list(range(dh // 128 - 1))
                for tt in range(nkc):
                    ps = wop.tile([128, min(din, 1024)], F32, tag="wops")
                    for i, kk in enumerate(kks):
                        lhsT = attT[kk][:, tt * 128:(tt + 1) * 128]
                        for n2 in range(0, din, 512):
                            nc.tensor.matmul(
                                ps[:, n2:n2 + 512], lhsT,
                                woT_sb[kk][:, n2:n2 + 512],
                                start=(i == 0), stop=(i == len(kks) - 1))
                    o = osb.tile([128, min(din, 1024)], F32, tag="outsb")
                    nc.scalar.copy(o[:], ps[:])
                    nc.sync.dma_start(out_d[tt * 128:(tt + 1) * 128, :], o[:])

    nc.compile()
    return nc


_PROG = None


def _get_program():
    global _PROG
    if _PROG is None:
        _PROG = build_program()
    return _PROG


def _rope_perm(nhc):
    """Per-head row permutation deinterleaving (re, im) pairs."""
    p = []
    for h in range(nhc):
        base = h * HEAD_DIM
        p.extend(base + 2 * i for i in range(HEAD_DIM // 2))
        p.extend(base + 2 * i + 1 for i in range(HEAD_DIM // 2))
    return np.array(p)


def make_core_inputs(x, position_ids, wq, wk, wv, wo, freqs_cos, freqs_sin):
    """Shard + pre-layout the full inputs for the 8 cores."""
    fc = np.asarray(freqs_cos, np.float32)
    fs = np.asarray(freqs_sin, np.float32)
    pos = np.asarray(position_ids)
    perm = _rope_perm(N_HEADS)
    wq_p = np.asarray(wq, np.float32)[perm]
    wk_p = np.asarray(wk, np.float32)[perm]
    wv_ = np.asarray(wv, np.float32)
    wo_ = np.asarray(wo, np.float32)

    in_maps = []
    for c in range(N_CORES):
        b, hh = c // 2, c % 2
        hs = slice(hh * 8 * HEAD_DIM, (hh + 1) * 8 * HEAD_DIM)
        cos_b = fc[pos[b]]                    # [T, 32]
        sin_b = fs[pos[b]]
        cosr = np.tile(cos_b.T, (4, 1))       # [128, T]
        sinr = np.tile(sin_b.T, (4, 1))
        in_maps.append({
            "xT": np.ascontiguousarray(
                np.asarray(x[b], np.float32).T).astype(NPBF16),
            "wqT": np.ascontiguousarray(wq_p[hs].T).astype(NPBF16),
            "wkT": np.ascontiguousarray(wk_p[hs].T).astype(NPBF16),
            "wvT": np.ascontiguousarray(wv_[hs].T).astype(NPBF16),
            "woT": np.ascontiguousarray(wo_[:, hs].T).astype(NPBF16),
            "cosr": cosr.astype(NPBF16),
            "sinp": sinr.astype(NPBF16),
            "sinn": (-sinr).astype(NPBF16),
        })
    return in_maps


def kernel(x, position_ids, mask, wq, wk, wv, wo, freqs_cos, freqs_sin,
           trace=False):
    nc = _get_program()
    in_maps = make_core_inputs(x, position_ids, wq, wk, wv, wo,
                               freqs_cos, freqs_sin)
    res = run_bass_kernel_spmd(nc, in_maps, list(range(N_CORES)),
                               trace=trace, trace_cores=[0] if trace else None)
    outs = [res.results[c]["out"] for c in range(N_CORES)]
    full = np.stack([outs[2 * b] + outs[2 * b + 1] for b in range(B)])
    kernel.last_results = res
    return full.astype(np.float32)
